# revision 15
# baseline (speedup 1.0000x reference)
"""Paged-attention GQA decode kernel for 8 Trainium2 NeuronCores (v5).

Problem: vLLM-style single-token decode with a paged KV cache.
  B=64 seqs, H=32 q heads, KVH=8 kv heads (GQA group G=4), D=128.
  out[b] = softmax(q.K^T/sqrt(D)) V over the first context_lens[b]+1
  tokens (new k/v inserted at position context_lens[b]).

Strategy (tensor-parallel over KV heads, no collectives):
  - Core c owns kv head c for ALL 64 sequences -> identical schedule on
    every core, exact per-seq lengths (chunk-of-128 rounding only).
  - Mixed precision: sequences with len >= 256 store K and V as fp8
    (e3m4); the few shorter ones as bf16 (short sequences are the
    dominant per-seq quantization-error terms, and upgrading them costs
    almost nothing in bytes).
  - Host packs per-core chunk-aligned streams (seqs sorted by len desc,
    fp8 segment is a prefix), zero-padded tails.
  - Device dataflow per 128-token chunk (both matmuls put the BIG
    operand in the stationary ldweights path, which loads ~2 rows/clk,
    and stream only 4 columns):
      QK: lhsT=K-chunk [D=128, 128] (ldw), rhs=q [D, G=4] -> st[128, G]
      exp(st) -> W [128, G] bf16 (resident store, indexed by chunk)
      PV: lhsT=V-chunk [128 tok, 128 dims] (ldw), rhs=W [128, G]
          -> accumulate o[dims=128, G] per sequence in PSUM.
    This is ~72ns/chunk vs ~100ns for the v4 orientation that streamed
    V's 129 columns.
  - Softmax denominators: matmul(lhsT=W-columns [128, <=128], rhs=ones
    [128, 1]) -> per-(chunk,g) column sums.  Chunk padding contributes
    exp(0)=1 per pad token (K pad columns are zero), which the host
    subtracts exactly (denom -= n_pad) before the final division.
  - K stream DMAs ride the Sync engine queue, V stream DMAs the Vector
    engine queue (two HW DGE rings pull from HBM in parallel).
  - Final division happens on host (64x512 divide).
"""

import sys

if "/opt/trn_rl_repo" not in sys.path:
    sys.path.insert(0, "/opt/trn_rl_repo")

from contextlib import ExitStack

import numpy as np
import ml_dtypes

import concourse.bass as bass
import concourse.tile as tile
from concourse import mybir
from concourse.bass_utils import run_bass_kernel_spmd

B, H, KVH, D = 64, 32, 8, 128
G = H // KVH                      # 4
BS, MB = 16, 128
NB = B * MB                       # 8192
L = MB * BS                       # 2048
SCALE = 0.08838834764831845
NCORES = 8
NPAIRS = B                        # 64 sequences = pairs per core (1 kvh each)
CHUNK = 128                       # token chunk
EXPB = 64                         # max chunks per batch (exp + DMA stage)
RAMP = 16                         # first-batch width (starts compute early)
FP8_MIN_LEN = 256                 # len >= this => K,V stored fp8
WARMUP = 16                       # PE clock-ramp dummy matmuls

BF16 = mybir.dt.bfloat16
FP8 = mybir.dt.float8e3
F32 = mybir.dt.float32
NP_BF16 = ml_dtypes.bfloat16
NP_FP8 = ml_dtypes.float8_e3m4

LAST_RESULTS = None


def _batch_plan(nch, psplit):
    """Return (cstart, batches) where batches = list of (c0, w, is_fp8).
    Chunks [0, c8) are the fp8 segment, [c8, totch) bf16."""
    cstart = np.concatenate([[0], np.cumsum(nch)]).astype(int)
    totch = int(cstart[-1])
    c8 = int(cstart[psplit])
    batches = []
    for seg0, seg1, is8 in ((0, c8, True), (c8, totch, False)):
        c = seg0
        first = True
        while c < seg1:
            w = min(RAMP if (first and seg0 == 0) else EXPB, seg1 - c)
            batches.append((c, w, is8))
            c += w
            first = False
    return cstart, totch, c8, batches


def _build(nc: bass.Bass, nch: list[int], psplit: int):
    assert len(nch) == NPAIRS
    cstart, totch, c8, batches = _batch_plan(nch, psplit)
    nbatch = len(batches)
    owner = np.empty(totch, dtype=int)
    for p in range(NPAIRS):
        owner[cstart[p] : cstart[p + 1]] = p
    cb = np.empty(totch, dtype=int)   # chunk -> batch index
    for gi, (c0, w, _) in enumerate(batches):
        cb[c0 : c0 + w] = gi
    pend = [int(cb[cstart[p + 1] - 1]) for p in range(NPAIRS)]
    span = max(
        int(cb[cstart[p + 1] - 1]) - int(cb[cstart[p]]) for p in range(NPAIRS)
    )
    # pairs grouped by the batch in which their last chunk lives
    groups = {}
    for p in range(NPAIRS):
        groups.setdefault(pend[p], []).append(p)
    # denominator matmuls: per batch, W-column ranges of <=128 cols
    dn_ranges = []                   # (batch, col0, ncols, dn_idx)
    for gi, (c0, w, _) in enumerate(batches):
        col0 = c0 * G
        end = (c0 + w) * G
        while col0 < end:
            ncols = min(128, end - col0)
            dn_ranges.append((gi, col0, ncols, len(dn_ranges)))
            col0 += ncols
    ndm = len(dn_ranges)
    dn_by_batch = {}
    for gi, col0, ncols, di in dn_ranges:
        dn_by_batch.setdefault(gi, []).append((col0, ncols, di))

    kt8_d = v8_d = kt16_d = v16_d = None
    if c8 > 0:
        kt8_d = nc.dram_tensor("kt8", [D, c8 * CHUNK], FP8, kind="ExternalInput")
        v8_d = nc.dram_tensor("v8", [CHUNK, c8, D], FP8, kind="ExternalInput")
    if totch > c8:
        kt16_d = nc.dram_tensor(
            "kt16", [D, (totch - c8) * CHUNK], BF16, kind="ExternalInput"
        )
        v16_d = nc.dram_tensor(
            "v16", [CHUNK, totch - c8, D], BF16, kind="ExternalInput"
        )
    qt_d = nc.dram_tensor("qt", [D, NPAIRS * G], BF16, kind="ExternalInput")
    out_d = nc.dram_tensor("out", [D, NPAIRS * G], F32, kind="ExternalOutput")
    dn_d = nc.dram_tensor("dn", [CHUNK, ndm], F32, kind="ExternalOutput")

    with tile.TileContext(nc) as tc, ExitStack() as ctx:
        ktp = ctx.enter_context(tc.tile_pool(name="ktp", bufs=6))
        # all V tiles stay resident -> V DMAs never wait on tile reuse
        vp = ctx.enter_context(tc.tile_pool(name="vp", bufs=nbatch))
        stp = ctx.enter_context(tc.tile_pool(name="stp", bufs=3, space="PSUM"))
        otp = ctx.enter_context(tc.tile_pool(name="otp", bufs=3, space="PSUM"))
        dnp = ctx.enter_context(tc.tile_pool(name="dnp", bufs=1, space="PSUM"))
        singles = ctx.enter_context(tc.tile_pool(name="singles", bufs=1))

        # vtiles[g] = (tile, batch_c0); W store indexed by global chunk
        vtiles = {}
        qt_sb = None
        wt_sb = singles.tile([CHUNK, totch * G], BF16)
        ostage = singles.tile([D, NPAIRS * G], F32)
        ones_sb = singles.tile([CHUNK, 1], BF16)
        nc.gpsimd.memset(ones_sb, 1.0)
        dn_ps = dnp.tile([CHUNK, ndm], F32)
        dn_sb = singles.tile([CHUNK, ndm], F32)

        # PE warm-up: the HAM clock gate holds the tensor engine at 1.2GHz
        # until it sees ~3.4us of sustained matmul activity.  The first kt
        # DMA cannot land before ~8us (framework prologue + transfer), so
        # burn that dead time on dummy matmuls fed from a memset tile.
        warm_sb = singles.tile([CHUNK, CHUNK], BF16)
        nc.vector.memset(warm_sb, 0.0)
        warm_ps = ctx.enter_context(
            tc.tile_pool(name="warmp", bufs=1, space="PSUM")
        ).tile([G, CHUNK], F32)
        for _ in range(WARMUP):
            nc.tensor.matmul(
                out=warm_ps,
                lhsT=warm_sb[:, :G],
                rhs=warm_sb[:, :CHUNK],
                start=True,
                stop=True,
            )

        def emit_pv_group(gi):
            ps = groups[gi]
            o_ps = otp.tile([D, len(ps) * G], F32, tag="o", name="o_ps")
            for i, p in enumerate(ps):
                n = nch[p]
                for c in range(n):
                    ci = int(cstart[p]) + c
                    bg = int(cb[ci])
                    v_t, bc0 = vtiles[bg]
                    nc.tensor.matmul(
                        out=o_ps[:, i * G : (i + 1) * G],
                        lhsT=v_t[:, ci - bc0, :],
                        rhs=wt_sb[:, ci * G : (ci + 1) * G],
                        start=(c == 0),
                        stop=(c == n - 1),
                    )
            p0, p1 = ps[0], ps[-1] + 1
            nc.scalar.copy(
                ostage[:, p0 * G : p1 * G], o_ps[:, : len(ps) * G]
            )
            nc.scalar.dma_start(
                out=out_d[:, p0 * G : p1 * G],
                in_=ostage[:, p0 * G : p1 * G],
            )

        def emit_dn(gi):
            for col0, ncols, di in dn_by_batch.get(gi, []):
                nc.tensor.matmul(
                    out=dn_ps[:ncols, di : di + 1],
                    lhsT=wt_sb[:, col0 : col0 + ncols],
                    rhs=ones_sb,
                    start=True,
                    stop=True,
                )

        # ---- post ALL stream DMAs upfront ----
        # kt+qt ride the Sync DGE ring, v the Scalar ring.  The v posts
        # have no dependencies (all tiles resident), so that ring free-runs;
        # kt posts beyond `ktp` bufs wait on QK completion via pool reuse
        # but never sit behind compute-dependent instructions.  Keeping the
        # out DMAs on Scalar (after their copies) keeps both stream rings
        # unblocked.
        kttiles = {}
        for g, (c0, w, is8) in enumerate(batches):
            kdt, vdt = (FP8, FP8) if is8 else (BF16, BF16)
            k_d, v_d = (kt8_d, v8_d) if is8 else (kt16_d, v16_d)
            s0 = c0 if is8 else c0 - c8   # chunk offset within segment
            kt_t = ktp.tile([D, w * CHUNK], kdt, tag="kt", name="kt_t")
            nc.sync.dma_start(
                out=kt_t,
                in_=k_d[:, s0 * CHUNK : (s0 + w) * CHUNK],
            )
            kttiles[g] = kt_t
            if qt_sb is None:
                qt_sb = singles.tile([D, NPAIRS * G], BF16)
                nc.sync.dma_start(out=qt_sb, in_=qt_d[:, :])
            v_t = vp.tile([CHUNK, w, D], vdt, tag="v", name="v_t")
            nc.scalar.dma_start(
                out=v_t,
                in_=v_d[:, s0 : s0 + w, :],
            )
            vtiles[g] = (v_t, c0)

        for g, (c0, w, is8) in enumerate(batches):
            kt_t = kttiles[g]
            st_ps = stp.tile([CHUNK, w * G], F32, tag="st", name="st_ps")
            for j in range(w):
                ci = c0 + j
                p = int(owner[ci])
                nc.tensor.matmul(
                    out=st_ps[:, j * G : (j + 1) * G],
                    lhsT=kt_t[:, j * CHUNK : (j + 1) * CHUNK],
                    rhs=qt_sb[:, p * G : (p + 1) * G],
                    start=True,
                    stop=True,
                )
            # PV + denominators for work that became ready after the
            # previous batch's exp
            if groups.get(g - 1):
                emit_pv_group(g - 1)
            emit_dn(g - 1)
            nc.scalar.activation(
                out=wt_sb[:, c0 * G : (c0 + w) * G],
                in_=st_ps[:, : w * G],
                func=mybir.ActivationFunctionType.Exp,
            )

        if groups.get(nbatch - 1):
            emit_pv_group(nbatch - 1)
        emit_dn(nbatch - 1)
        nc.scalar.copy(dn_sb, dn_ps)
        nc.scalar.dma_start(out=dn_d[:, :], in_=dn_sb)

    _split_excess_waits(nc)
    _coalesce_pe_updates(nc)
    return cstart, totch, c8, dn_ranges


def _split_excess_waits(nc: bass.Bass):
    """Walrus can encode only one sync wait per TPB instruction.  Move the
    extras onto standalone EventSemaphore instructions inserted just before,
    on the same engine queue."""
    for fn in nc.m.functions:
        for bb in fn.blocks:
            insts = bb.instructions
            out = []
            changed = False
            for inst in insts:
                si = inst.sync_info
                if (
                    not isinstance(inst, mybir.InstEventSemaphore)
                    and si is not None
                    and si.on_wait
                    and len(si.on_wait) > 1
                ):
                    waits = list(si.on_wait)
                    for k, w in enumerate(waits[:-1]):
                        out.append(
                            mybir.InstEventSemaphore(
                                name=f"{inst.name}-w{k}",
                                engine=inst.engine,
                                ins=[],
                                outs=[],
                                sync_info=mybir.SyncInfo(on_wait=[w], on_update=[]),
                            )
                        )
                    inst.sync_info = mybir.SyncInfo(
                        on_wait=[waits[-1]], on_update=list(si.on_update or [])
                    )
                    changed = True
                out.append(inst)
            if changed:
                bb.instructions = out


def kernel(q, k, v, k_cache, v_cache, block_tables, context_lens, trace=False):
    global LAST_RESULTS
    q = np.asarray(q, dtype=np.float32)
    k = np.asarray(k, dtype=np.float32)
    v = np.asarray(v, dtype=np.float32)
    k_cache = np.asarray(k_cache, dtype=np.float32)
    v_cache = np.asarray(v_cache, dtype=np.float32)
    block_tables = np.asarray(block_tables)
    context_lens = np.asarray(context_lens)

    lens = context_lens.astype(np.int64) + 1  # valid tokens incl. new one

    # ---- dense gather of the paged cache: [B, L, KVH, D] ----
    ident = np.array_equal(
        block_tables, np.arange(B * MB, dtype=block_tables.dtype).reshape(B, MB)
    )
    if ident:
        kd = k_cache.reshape(B, L, KVH, D)
        vd = v_cache.reshape(B, L, KVH, D)
    else:
        bt = block_tables.astype(np.int64).reshape(-1)
        kd = k_cache.reshape(NB, BS, KVH, D)[bt].reshape(B, L, KVH, D)
        vd = v_cache.reshape(NB, BS, KVH, D)[bt].reshape(B, L, KVH, D)

    kh = k.reshape(B, KVH, D)
    vh = v.reshape(B, KVH, D)

    # ---- sort by length desc, chunk-aligned packing, fp8 prefix ----
    order = np.argsort(-lens, kind="stable")
    lens_s = lens[order]
    nch = [(int(s) + CHUNK - 1) // CHUNK for s in lens_s]
    psplit = int(np.sum(lens_s >= FP8_MIN_LEN))
    cstart, totch, c8, _batches = _batch_plan(nch, psplit)

    kt8 = np.zeros((KVH, D, c8 * CHUNK), dtype=NP_FP8)
    v8 = np.zeros((KVH, CHUNK, c8, D), dtype=NP_FP8)
    kt16 = np.zeros((KVH, D, (totch - c8) * CHUNK), dtype=NP_BF16)
    v16 = np.zeros((KVH, CHUNK, totch - c8, D), dtype=NP_BF16)
    for p in range(NPAIRS):
        b = int(order[p])
        ln = int(lens_s[p])
        n = nch[p]
        ks = np.zeros((n * CHUNK, KVH, D), dtype=np.float32)
        ks[: ln - 1] = kd[b, : ln - 1]
        ks[ln - 1] = kh[b]
        vs = np.zeros((n * CHUNK, KVH, D), dtype=np.float32)
        vs[: ln - 1] = vd[b, : ln - 1]
        vs[ln - 1] = vh[b]
        ktp = ks.transpose(1, 2, 0)                                  # [KVH,D,n*128]
        vxp = vs.reshape(n, CHUNK, KVH, D).transpose(2, 1, 0, 3)     # [KVH,128,n,D]
        s = int(cstart[p])
        if p < psplit:
            kt8[:, :, s * CHUNK : (s + n) * CHUNK] = ktp.astype(NP_FP8)
            v8[:, :, s : s + n, :] = vxp.astype(NP_FP8)
        else:
            s -= c8
            kt16[:, :, s * CHUNK : (s + n) * CHUNK] = ktp.astype(NP_BF16)
            v16[:, :, s : s + n, :] = vxp.astype(NP_BF16)

    qh = (q.reshape(B, KVH, G, D) * SCALE)[order]          # [64, KVH, G, D]
    qt_all = np.ascontiguousarray(
        qh.transpose(1, 3, 0, 2).reshape(KVH, D, NPAIRS * G)
    ).astype(NP_BF16)

    in_maps = []
    for c in range(NCORES):
        m = {"qt": qt_all[c]}
        if c8 > 0:
            m["kt8"] = kt8[c]
            m["v8"] = v8[c]
        if totch > c8:
            m["kt16"] = kt16[c]
            m["v16"] = v16[c]
        in_maps.append(m)

    nc = bass.Bass("TRN2")
    _, _, _, dn_ranges = _build(nc, nch, psplit)

    res = run_bass_kernel_spmd(
        nc, in_maps, core_ids=list(range(NCORES)), trace=trace
    )
    LAST_RESULTS = res

    # ---- host: assemble denominators and divide ----
    # dn[:, di] = column sums of W cols [col0, col0+ncols); W col of
    # (chunk ci, g) is ci*G+g.  Chunk padding contributes exp(0)=1 per pad
    # token; subtract it exactly.
    col_of = np.full(totch * G, -1, dtype=np.int64)   # wcol -> (di, row)
    row_of = np.full(totch * G, -1, dtype=np.int64)
    for _, col0, ncols, di in dn_ranges:
        col_of[col0 : col0 + ncols] = di
        row_of[col0 : col0 + ncols] = np.arange(ncols)

    out = np.empty((B, KVH, G, D), dtype=np.float32)
    for c in range(NCORES):
        r = np.asarray(res.results[c]["out"], dtype=np.float32)       # [D, 64*G]
        dn = np.asarray(res.results[c]["dn"], dtype=np.float32)       # [128, ndm]
        for p in range(NPAIRS):
            b = int(order[p])
            n = nch[p]
            npad = n * CHUNK - int(lens_s[p])
            for g_ in range(G):
                wcols = (np.arange(cstart[p], cstart[p] + n)) * G + g_
                denom = dn[row_of[wcols], col_of[wcols]].sum() - npad
                out[b, c, g_] = r[:, p * G + g_] / denom
    return out.reshape(B, H * D)


def _coalesce_pe_updates(nc: bass.Bass):
    """Merge per-matmul semaphore increments.  Tile emits a sem-inc(1) on
    every matmul so cross-engine waiters can count producers, but EVT_SEM
    register writes serialize on the PE sequencer (~26ns each), pacing the
    whole tensor queue.  Matmuls complete in pc order, so within a run of
    wait-free PE instructions the increments can ride on the run's last
    carrier for each semaphore with the summed value: every waiter sees the
    same final count, no earlier."""
    MM = mybir.InstMatmult
    LDW = mybir.InstLdweights
    for fn in nc.m.functions:
        for bb in fn.blocks:
            carriers = []

            def flush():
                if len(carriers) < 2:
                    carriers.clear()
                    return
                per_sem = {}
                for inst in carriers:
                    for u in inst.sync_info.on_update:
                        per_sem.setdefault(u.id, []).append((inst, u))
                for sid, lst in per_sem.items():
                    if len(lst) < 2:
                        continue
                    total = sum(u.update_value for _, u in lst)
                    for inst, u in lst[:-1]:
                        si = inst.sync_info
                        ups = [x for x in si.on_update if x is not u]
                        inst.sync_info = mybir.SyncInfo(
                            on_wait=list(si.on_wait or []), on_update=ups
                        )
                    last_inst, last_u = lst[-1]
                    si = last_inst.sync_info
                    ups = [
                        x
                        if x is not last_u
                        else mybir.SyncUpdate(
                            sync_type=x.sync_type,
                            id=x.id,
                            ant_name=x.ant_name,
                            update_mode="sem-add-imm",
                            update_value=total,
                            update_reg=None,
                        )
                        for x in si.on_update
                    ]
                    last_inst.sync_info = mybir.SyncInfo(
                        on_wait=list(si.on_wait or []), on_update=ups
                    )
                carriers.clear()

            for inst in bb.instructions:
                if str(inst.engine) != "EngineType.PE":
                    continue
                si = inst.sync_info
                if si is not None and si.on_wait:
                    flush()
                if isinstance(inst, MM):
                    ups = (si.on_update if si is not None else None) or []
                    if ups and all(
                        u.sync_type == "semaphore"
                        and u.update_mode == "sem-inc"
                        and u.update_reg is None
                        and isinstance(u.update_value, int)
                        for u in ups
                    ):
                        carriers.append(inst)
                elif isinstance(inst, LDW):
                    if si is not None and si.on_update:
                        flush()
                else:
                    flush()
            flush()


# revision 17
# speedup vs baseline: 1.1733x; 1.1733x over previous
"""Paged-attention GQA decode kernel for 8 Trainium2 NeuronCores (v5).

Problem: vLLM-style single-token decode with a paged KV cache.
  B=64 seqs, H=32 q heads, KVH=8 kv heads (GQA group G=4), D=128.
  out[b] = softmax(q.K^T/sqrt(D)) V over the first context_lens[b]+1
  tokens (new k/v inserted at position context_lens[b]).

Strategy (tensor-parallel over KV heads, no collectives):
  - Core c owns kv head c for ALL 64 sequences -> identical schedule on
    every core, exact per-seq lengths (chunk-of-128 rounding only).
  - Mixed precision: sequences with len >= 256 store K and V as fp8
    (e3m4); the few shorter ones as bf16 (short sequences are the
    dominant per-seq quantization-error terms, and upgrading them costs
    almost nothing in bytes).
  - Host packs per-core chunk-aligned streams (seqs sorted by len desc,
    fp8 segment is a prefix), zero-padded tails.
  - Device dataflow per 128-token chunk (both matmuls put the BIG
    operand in the stationary ldweights path, which loads ~2 rows/clk,
    and stream only 4 columns):
      QK: lhsT=K-chunk [D=128, 128] (ldw), rhs=q [D, G=4] -> st[128, G]
      exp(st) -> W [128, G] bf16 (resident store, indexed by chunk)
      PV: lhsT=V-chunk [128 tok, 128 dims] (ldw), rhs=W [128, G]
          -> accumulate o[dims=128, G] per sequence in PSUM.
    This is ~72ns/chunk vs ~100ns for the v4 orientation that streamed
    V's 129 columns.
  - Softmax denominators: matmul(lhsT=W-columns [128, <=128], rhs=ones
    [128, 1]) -> per-(chunk,g) column sums.  Chunk padding contributes
    exp(0)=1 per pad token (K pad columns are zero), which the host
    subtracts exactly (denom -= n_pad) before the final division.
  - K stream DMAs ride the Sync engine queue, V stream DMAs the Vector
    engine queue (two HW DGE rings pull from HBM in parallel).
  - Final division happens on host (64x512 divide).
"""

import sys

if "/opt/trn_rl_repo" not in sys.path:
    sys.path.insert(0, "/opt/trn_rl_repo")

from contextlib import ExitStack

import numpy as np
import ml_dtypes

import concourse.bass as bass
import concourse.tile as tile
from concourse import mybir
from concourse.bass_utils import run_bass_kernel_spmd

B, H, KVH, D = 64, 32, 8, 128
G = H // KVH                      # 4
BS, MB = 16, 128
NB = B * MB                       # 8192
L = MB * BS                       # 2048
SCALE = 0.08838834764831845
NCORES = 8
NPAIRS = B                        # 64 sequences = pairs per core (1 kvh each)
CHUNK = 128                       # token chunk
EXPB = 64                         # max chunks per batch (exp + DMA stage)
RAMP = 16                         # first-batch width (starts compute early)
FP8_MIN_LEN = 256                 # len >= this => K,V stored fp8
WARMUP = 16                       # PE clock-ramp dummy matmuls

BF16 = mybir.dt.bfloat16
FP8 = mybir.dt.float8e3
F32 = mybir.dt.float32
NP_BF16 = ml_dtypes.bfloat16
NP_FP8 = ml_dtypes.float8_e3m4

LAST_RESULTS = None


def _batch_plan(nch, psplit):
    """Return (cstart, batches) where batches = list of (c0, w, is_fp8).
    Chunks [0, c8) are the fp8 segment, [c8, totch) bf16."""
    cstart = np.concatenate([[0], np.cumsum(nch)]).astype(int)
    totch = int(cstart[-1])
    c8 = int(cstart[psplit])
    batches = []
    for seg0, seg1, is8 in ((0, c8, True), (c8, totch, False)):
        c = seg0
        first = True
        while c < seg1:
            w = min(RAMP if (first and seg0 == 0) else EXPB, seg1 - c)
            batches.append((c, w, is8))
            c += w
            first = False
    return cstart, totch, c8, batches


def _build(nc: bass.Bass, nch: list[int], psplit: int):
    assert len(nch) == NPAIRS
    cstart, totch, c8, batches = _batch_plan(nch, psplit)
    nbatch = len(batches)
    owner = np.empty(totch, dtype=int)
    for p in range(NPAIRS):
        owner[cstart[p] : cstart[p + 1]] = p
    cb = np.empty(totch, dtype=int)   # chunk -> batch index
    for gi, (c0, w, _) in enumerate(batches):
        cb[c0 : c0 + w] = gi
    pend = [int(cb[cstart[p + 1] - 1]) for p in range(NPAIRS)]
    span = max(
        int(cb[cstart[p + 1] - 1]) - int(cb[cstart[p]]) for p in range(NPAIRS)
    )
    # pairs grouped by the batch in which their last chunk lives
    groups = {}
    for p in range(NPAIRS):
        groups.setdefault(pend[p], []).append(p)
    # denominator matmuls: per batch, W-column ranges of <=128 cols
    dn_ranges = []                   # (batch, col0, ncols, dn_idx)
    for gi, (c0, w, _) in enumerate(batches):
        col0 = c0 * G
        end = (c0 + w) * G
        while col0 < end:
            ncols = min(128, end - col0)
            dn_ranges.append((gi, col0, ncols, len(dn_ranges)))
            col0 += ncols
    ndm = len(dn_ranges)
    dn_by_batch = {}
    for gi, col0, ncols, di in dn_ranges:
        dn_by_batch.setdefault(gi, []).append((col0, ncols, di))

    kt8_d = v8_d = kt16_d = v16_d = None
    if c8 > 0:
        kt8_d = nc.dram_tensor("kt8", [D, c8 * CHUNK], FP8, kind="ExternalInput")
        v8_d = nc.dram_tensor("v8", [CHUNK, c8, D], FP8, kind="ExternalInput")
    if totch > c8:
        kt16_d = nc.dram_tensor(
            "kt16", [D, (totch - c8) * CHUNK], BF16, kind="ExternalInput"
        )
        v16_d = nc.dram_tensor(
            "v16", [CHUNK, totch - c8, D], BF16, kind="ExternalInput"
        )
    qt_d = nc.dram_tensor("qt", [D, NPAIRS * G], BF16, kind="ExternalInput")
    out_d = nc.dram_tensor("out", [D, NPAIRS * G], F32, kind="ExternalOutput")
    dn_d = nc.dram_tensor("dn", [CHUNK, ndm], F32, kind="ExternalOutput")

    with tile.TileContext(nc) as tc, ExitStack() as ctx:
        ktp = ctx.enter_context(tc.tile_pool(name="ktp", bufs=6))
        # all V tiles stay resident -> V DMAs never wait on tile reuse
        vp = ctx.enter_context(tc.tile_pool(name="vp", bufs=nbatch))
        stp = ctx.enter_context(tc.tile_pool(name="stp", bufs=3, space="PSUM"))
        otp = ctx.enter_context(tc.tile_pool(name="otp", bufs=3, space="PSUM"))
        dnp = ctx.enter_context(tc.tile_pool(name="dnp", bufs=1, space="PSUM"))
        singles = ctx.enter_context(tc.tile_pool(name="singles", bufs=1))

        # vtiles[g] = (tile, batch_c0); W store indexed by global chunk
        vtiles = {}
        qt_sb = None
        wt_sb = singles.tile([CHUNK, totch * G], BF16)
        ostage = singles.tile([D, NPAIRS * G], F32)
        ones_sb = singles.tile([CHUNK, 1], BF16)
        nc.gpsimd.memset(ones_sb, 1.0)
        dn_ps = dnp.tile([CHUNK, ndm], F32)
        dn_sb = singles.tile([CHUNK, ndm], F32)

        # PE warm-up: the HAM clock gate holds the tensor engine at 1.2GHz
        # until it sees ~3.4us of sustained matmul activity.  The first kt
        # DMA cannot land before ~8us (framework prologue + transfer), so
        # burn that dead time on dummy matmuls fed from a memset tile.
        warm_sb = singles.tile([CHUNK, CHUNK], BF16)
        nc.vector.memset(warm_sb, 0.0)
        warm_ps = ctx.enter_context(
            tc.tile_pool(name="warmp", bufs=1, space="PSUM")
        ).tile([G, CHUNK], F32)
        for _ in range(WARMUP):
            nc.tensor.matmul(
                out=warm_ps,
                lhsT=warm_sb[:, :G],
                rhs=warm_sb[:, :CHUNK],
                start=True,
                stop=True,
            )

        def emit_pv_group(gi):
            ps = groups[gi]
            o_ps = otp.tile([D, len(ps) * G], F32, tag="o", name="o_ps")
            for i, p in enumerate(ps):
                n = nch[p]
                for c in range(n):
                    ci = int(cstart[p]) + c
                    bg = int(cb[ci])
                    v_t, bc0 = vtiles[bg]
                    nc.tensor.matmul(
                        out=o_ps[:, i * G : (i + 1) * G],
                        lhsT=v_t[:, ci - bc0, :],
                        rhs=wt_sb[:, ci * G : (ci + 1) * G],
                        start=(c == 0),
                        stop=(c == n - 1),
                    )
            p0, p1 = ps[0], ps[-1] + 1
            nc.scalar.copy(
                ostage[:, p0 * G : p1 * G], o_ps[:, : len(ps) * G]
            )
            nc.scalar.dma_start(
                out=out_d[:, p0 * G : p1 * G],
                in_=ostage[:, p0 * G : p1 * G],
            )

        def emit_dn(gi):
            for col0, ncols, di in dn_by_batch.get(gi, []):
                nc.tensor.matmul(
                    out=dn_ps[:ncols, di : di + 1],
                    lhsT=wt_sb[:, col0 : col0 + ncols],
                    rhs=ones_sb,
                    start=True,
                    stop=True,
                )

        # ---- post ALL stream DMAs upfront, all on the Sync DGE ring ----
        # A single HW ring already sustains the per-core HBM ceiling
        # (~420 GB/s), so there is nothing to gain from a second ring, and
        # putting stream DMAs on the Scalar queue is fatal: ring-full
        # backpressure blocks the posting instruction, and everything
        # behind it (exp, copies) stalls.  Sync carries nothing but the
        # k/v/q stream posts; Scalar keeps exp + copies + out DMAs.
        kttiles = {}
        for g, (c0, w, is8) in enumerate(batches):
            kdt, vdt = (FP8, FP8) if is8 else (BF16, BF16)
            k_d, v_d = (kt8_d, v8_d) if is8 else (kt16_d, v16_d)
            s0 = c0 if is8 else c0 - c8   # chunk offset within segment
            kt_t = ktp.tile([D, w * CHUNK], kdt, tag="kt", name="kt_t")
            nc.sync.dma_start(
                out=kt_t,
                in_=k_d[:, s0 * CHUNK : (s0 + w) * CHUNK],
            )
            kttiles[g] = kt_t
            if qt_sb is None:
                qt_sb = singles.tile([D, NPAIRS * G], BF16)
                nc.sync.dma_start(out=qt_sb, in_=qt_d[:, :])
            v_t = vp.tile([CHUNK, w, D], vdt, tag="v", name="v_t")
            nc.sync.dma_start(
                out=v_t,
                in_=v_d[:, s0 : s0 + w, :],
            )
            vtiles[g] = (v_t, c0)

        for g, (c0, w, is8) in enumerate(batches):
            kt_t = kttiles[g]
            st_ps = stp.tile([CHUNK, w * G], F32, tag="st", name="st_ps")
            for j in range(w):
                ci = c0 + j
                p = int(owner[ci])
                nc.tensor.matmul(
                    out=st_ps[:, j * G : (j + 1) * G],
                    lhsT=kt_t[:, j * CHUNK : (j + 1) * CHUNK],
                    rhs=qt_sb[:, p * G : (p + 1) * G],
                    start=True,
                    stop=True,
                )
            # PV + denominators for work that became ready after the
            # previous batch's exp
            if groups.get(g - 1):
                emit_pv_group(g - 1)
            emit_dn(g - 1)
            nc.scalar.activation(
                out=wt_sb[:, c0 * G : (c0 + w) * G],
                in_=st_ps[:, : w * G],
                func=mybir.ActivationFunctionType.Exp,
            )

        if groups.get(nbatch - 1):
            emit_pv_group(nbatch - 1)
        emit_dn(nbatch - 1)
        nc.scalar.copy(dn_sb, dn_ps)
        nc.scalar.dma_start(out=dn_d[:, :], in_=dn_sb)

    _split_excess_waits(nc)
    _coalesce_pe_updates(nc)
    return cstart, totch, c8, dn_ranges


def _split_excess_waits(nc: bass.Bass):
    """Walrus can encode only one sync wait per TPB instruction.  Move the
    extras onto standalone EventSemaphore instructions inserted just before,
    on the same engine queue."""
    for fn in nc.m.functions:
        for bb in fn.blocks:
            insts = bb.instructions
            out = []
            changed = False
            for inst in insts:
                si = inst.sync_info
                if (
                    not isinstance(inst, mybir.InstEventSemaphore)
                    and si is not None
                    and si.on_wait
                    and len(si.on_wait) > 1
                ):
                    waits = list(si.on_wait)
                    for k, w in enumerate(waits[:-1]):
                        out.append(
                            mybir.InstEventSemaphore(
                                name=f"{inst.name}-w{k}",
                                engine=inst.engine,
                                ins=[],
                                outs=[],
                                sync_info=mybir.SyncInfo(on_wait=[w], on_update=[]),
                            )
                        )
                    inst.sync_info = mybir.SyncInfo(
                        on_wait=[waits[-1]], on_update=list(si.on_update or [])
                    )
                    changed = True
                out.append(inst)
            if changed:
                bb.instructions = out


def kernel(q, k, v, k_cache, v_cache, block_tables, context_lens, trace=False):
    global LAST_RESULTS
    q = np.asarray(q, dtype=np.float32)
    k = np.asarray(k, dtype=np.float32)
    v = np.asarray(v, dtype=np.float32)
    k_cache = np.asarray(k_cache, dtype=np.float32)
    v_cache = np.asarray(v_cache, dtype=np.float32)
    block_tables = np.asarray(block_tables)
    context_lens = np.asarray(context_lens)

    lens = context_lens.astype(np.int64) + 1  # valid tokens incl. new one

    # ---- dense gather of the paged cache: [B, L, KVH, D] ----
    ident = np.array_equal(
        block_tables, np.arange(B * MB, dtype=block_tables.dtype).reshape(B, MB)
    )
    if ident:
        kd = k_cache.reshape(B, L, KVH, D)
        vd = v_cache.reshape(B, L, KVH, D)
    else:
        bt = block_tables.astype(np.int64).reshape(-1)
        kd = k_cache.reshape(NB, BS, KVH, D)[bt].reshape(B, L, KVH, D)
        vd = v_cache.reshape(NB, BS, KVH, D)[bt].reshape(B, L, KVH, D)

    kh = k.reshape(B, KVH, D)
    vh = v.reshape(B, KVH, D)

    # ---- sort by length desc, chunk-aligned packing, fp8 prefix ----
    order = np.argsort(-lens, kind="stable")
    lens_s = lens[order]
    nch = [(int(s) + CHUNK - 1) // CHUNK for s in lens_s]
    psplit = int(np.sum(lens_s >= FP8_MIN_LEN))
    cstart, totch, c8, _batches = _batch_plan(nch, psplit)

    kt8 = np.zeros((KVH, D, c8 * CHUNK), dtype=NP_FP8)
    v8 = np.zeros((KVH, CHUNK, c8, D), dtype=NP_FP8)
    kt16 = np.zeros((KVH, D, (totch - c8) * CHUNK), dtype=NP_BF16)
    v16 = np.zeros((KVH, CHUNK, totch - c8, D), dtype=NP_BF16)
    for p in range(NPAIRS):
        b = int(order[p])
        ln = int(lens_s[p])
        n = nch[p]
        ks = np.zeros((n * CHUNK, KVH, D), dtype=np.float32)
        ks[: ln - 1] = kd[b, : ln - 1]
        ks[ln - 1] = kh[b]
        vs = np.zeros((n * CHUNK, KVH, D), dtype=np.float32)
        vs[: ln - 1] = vd[b, : ln - 1]
        vs[ln - 1] = vh[b]
        ktp = ks.transpose(1, 2, 0)                                  # [KVH,D,n*128]
        vxp = vs.reshape(n, CHUNK, KVH, D).transpose(2, 1, 0, 3)     # [KVH,128,n,D]
        s = int(cstart[p])
        if p < psplit:
            kt8[:, :, s * CHUNK : (s + n) * CHUNK] = ktp.astype(NP_FP8)
            v8[:, :, s : s + n, :] = vxp.astype(NP_FP8)
        else:
            s -= c8
            kt16[:, :, s * CHUNK : (s + n) * CHUNK] = ktp.astype(NP_BF16)
            v16[:, :, s : s + n, :] = vxp.astype(NP_BF16)

    qh = (q.reshape(B, KVH, G, D) * SCALE)[order]          # [64, KVH, G, D]
    qt_all = np.ascontiguousarray(
        qh.transpose(1, 3, 0, 2).reshape(KVH, D, NPAIRS * G)
    ).astype(NP_BF16)

    in_maps = []
    for c in range(NCORES):
        m = {"qt": qt_all[c]}
        if c8 > 0:
            m["kt8"] = kt8[c]
            m["v8"] = v8[c]
        if totch > c8:
            m["kt16"] = kt16[c]
            m["v16"] = v16[c]
        in_maps.append(m)

    nc = bass.Bass("TRN2")
    _, _, _, dn_ranges = _build(nc, nch, psplit)

    res = run_bass_kernel_spmd(
        nc, in_maps, core_ids=list(range(NCORES)), trace=trace
    )
    LAST_RESULTS = res

    # ---- host: assemble denominators and divide ----
    # dn[:, di] = column sums of W cols [col0, col0+ncols); W col of
    # (chunk ci, g) is ci*G+g.  Chunk padding contributes exp(0)=1 per pad
    # token; subtract it exactly.
    col_of = np.full(totch * G, -1, dtype=np.int64)   # wcol -> (di, row)
    row_of = np.full(totch * G, -1, dtype=np.int64)
    for _, col0, ncols, di in dn_ranges:
        col_of[col0 : col0 + ncols] = di
        row_of[col0 : col0 + ncols] = np.arange(ncols)

    out = np.empty((B, KVH, G, D), dtype=np.float32)
    for c in range(NCORES):
        r = np.asarray(res.results[c]["out"], dtype=np.float32)       # [D, 64*G]
        dn = np.asarray(res.results[c]["dn"], dtype=np.float32)       # [128, ndm]
        for p in range(NPAIRS):
            b = int(order[p])
            n = nch[p]
            npad = n * CHUNK - int(lens_s[p])
            for g_ in range(G):
                wcols = (np.arange(cstart[p], cstart[p] + n)) * G + g_
                denom = dn[row_of[wcols], col_of[wcols]].sum() - npad
                out[b, c, g_] = r[:, p * G + g_] / denom
    return out.reshape(B, H * D)


def _coalesce_pe_updates(nc: bass.Bass):
    """Merge per-matmul semaphore increments.  Tile emits a sem-inc(1) on
    every matmul so cross-engine waiters can count producers, but EVT_SEM
    register writes serialize on the PE sequencer (~26ns each), pacing the
    whole tensor queue.  Matmuls complete in pc order, so within a run of
    wait-free PE instructions the increments can ride on the run's last
    carrier for each semaphore with the summed value: every waiter sees the
    same final count, no earlier."""
    MM = mybir.InstMatmult
    LDW = mybir.InstLdweights
    for fn in nc.m.functions:
        for bb in fn.blocks:
            carriers = []

            def flush():
                if len(carriers) < 2:
                    carriers.clear()
                    return
                per_sem = {}
                for inst in carriers:
                    for u in inst.sync_info.on_update:
                        per_sem.setdefault(u.id, []).append((inst, u))
                for sid, lst in per_sem.items():
                    if len(lst) < 2:
                        continue
                    total = sum(u.update_value for _, u in lst)
                    for inst, u in lst[:-1]:
                        si = inst.sync_info
                        ups = [x for x in si.on_update if x is not u]
                        inst.sync_info = mybir.SyncInfo(
                            on_wait=list(si.on_wait or []), on_update=ups
                        )
                    last_inst, last_u = lst[-1]
                    si = last_inst.sync_info
                    ups = [
                        x
                        if x is not last_u
                        else mybir.SyncUpdate(
                            sync_type=x.sync_type,
                            id=x.id,
                            ant_name=x.ant_name,
                            update_mode="sem-add-imm",
                            update_value=total,
                            update_reg=None,
                        )
                        for x in si.on_update
                    ]
                    last_inst.sync_info = mybir.SyncInfo(
                        on_wait=list(si.on_wait or []), on_update=ups
                    )
                carriers.clear()

            for inst in bb.instructions:
                if str(inst.engine) != "EngineType.PE":
                    continue
                si = inst.sync_info
                if si is not None and si.on_wait:
                    flush()
                if isinstance(inst, MM):
                    ups = (si.on_update if si is not None else None) or []
                    if ups and all(
                        u.sync_type == "semaphore"
                        and u.update_mode == "sem-inc"
                        and u.update_reg is None
                        and isinstance(u.update_value, int)
                        for u in ups
                    ):
                        carriers.append(inst)
                elif isinstance(inst, LDW):
                    if si is not None and si.on_update:
                        flush()
                else:
                    flush()
            flush()
